# revision 36
# baseline (speedup 1.0000x reference)
"""Trainium2 Bass kernel for nn_AttentionModel_47983374631276.

SDPA attention: B=2, H=16, S=2048, D=128, fp8-representable q/k/v with
per-tensor dequant scales (qs, ks, vs).

Sharding: batch*heads = 32 pairs -> 4 heads per core across 8 cores.
Each core runs its full S x S attention locally; no cross-core comm.

Per-head device algorithm:
  1. matmul1 in fp8e4 with DoubleRow perf mode (lossless: q/k values are
     exactly fp8-representable; D=128 contraction is split into 2 k-tiles
     of 64 partitions, PE processes both per pass at 0.5 cycles/row):
     logits^T[k, q] slices [128, 512] into PSUM f32.
  2. exp, split across three engines to beat the ACT-only roofline:
     - ACT (ScalarE): exact exp -> fp16, scale=qs*ks/sqrt(D), bias=-shift.
     - DVE + Pool: Schraudolph bit trick: one tensor_scalar
       (logit * A1024 + B1024) -> uint16 (round-nearest, saturating at 0)
       whose bits ARE the fp16 representation of exp(scale*l - shift)
       with a zero-mean multiplicative error of std 1.9%. ~44% of slices
       use the approximation; measured end-to-end rel err ~1.2e-2 < 2e-2.
  3. matmul2 (fp16): out[q, 129] = sum_kt P'^T[kt].T @ [V*vs | 1]
     (ones column gives the softmax denominator; V is pre-scaled by vs on
     the host - exactly representable in fp16 up to 2^-11 relative).
  4. evac: DVE reciprocal of the denominator, Pool tensor_scalar multiply,
     staged [128, 4, 128] f32 in SBUF, one DMA per 512 queries.

Software pipelining: phase2 of head h-1 is emitted after phase1 of head h
in a lower priority band, so the Tile scheduler fills PE gaps (ps1 ring
full) with matmul2 work while ACT/DVE/Pool drain exp chunks.
"""

import math
import os

import numpy as np
import ml_dtypes

import concourse.bacc as bacc
import concourse.bass as bass
import concourse.tile as tile
import concourse.mybir as mybir
from concourse.bass_utils import run_bass_kernel_spmd

N_CORES = 8
HEADS_PER_CORE = 4
S = 2048
D = 128
P = 128            # partitions
KT = S // P        # 16 k tiles per head
QB = 4             # q blocks of 512
QW = S // QB       # 512
VW = 130           # v row width: 128 values + ones col + pad

FP8 = mybir.dt.float8e4
BF16 = mybir.dt.bfloat16
FP16 = mybir.dt.float16
U16 = mybir.dt.uint16
F32 = mybir.dt.float32
MULT = mybir.AluOpType.mult
ADD = mybir.AluOpType.add

# exp chunk -> engine pattern per q-block (8 chunks of 2 kt each).
# 'A' = ScalarE exact exp, 'D' = DVE bit trick. (Pool/GPSIMD cannot read
# PSUM on trn2, so it can't join the exp work directly.)
ENG_PATTERN = [
    ("A", "A", "D", "A", "D", "A", "D", "A"),   # 5A 3D
    ("D", "A", "D", "A", "A", "D", "A", "D"),   # 4A 4D
]

# Schraudolph constant tuned so E[approx/exact] = 1 over the mantissa
# interval (zero-mean multiplicative error; bias would not cancel in rows
# that mix exact and approximated slices).
C0_SCHRAUDOLPH = -0.0564

# Stash of the most recent run results / program for test harnesses.
LAST_RESULTS = None
LAST_NC = None


def _build_program(c_scale: float, c_shift: float):
    nc = bacc.Bacc()

    q8_d = nc.dram_tensor("q8", [HEADS_PER_CORE, 64, 2, S], FP8, kind="ExternalInput")
    k8_d = nc.dram_tensor("k8", [HEADS_PER_CORE, 64, 2, S], FP8, kind="ExternalInput")
    v_d = nc.dram_tensor("v16", [HEADS_PER_CORE, P, KT, VW], FP16, kind="ExternalInput")
    out_d = nc.dram_tensor("out", [HEADS_PER_CORE, S, D], F32, kind="ExternalOutput")

    a1024 = float(np.float32(c_scale * math.log2(math.e) * 1024.0))
    b1024 = float(np.float32(
        (-c_shift * math.log2(math.e) + 15.0 + C0_SCHRAUDOLPH) * 1024.0
    ))

    with tile.TileContext(nc) as tc:
        with (
            tc.tile_pool(name="io", bufs=2) as io_pool,
            tc.tile_pool(name="ptp", bufs=8) as pt_pool,
            tc.tile_pool(name="outp", bufs=4) as out_pool,
            tc.tile_pool(name="stagep", bufs=4) as stage_pool,
            tc.tile_pool(name="smallp", bufs=4) as small_pool,
            tc.tile_pool(name="ps1p", bufs=3, space="PSUM") as ps1_pool,
            tc.tile_pool(name="ps2p", bufs=2, space="PSUM") as ps2_pool,
        ):
            P1_BAND = 0
            P2_BAND = 10_000_000
            HEAD_STRIDE = 100_000

            def emit_load(h):
                tc.cur_priority = P1_BAND + h * HEAD_STRIDE
                k8_sb = io_pool.tile([64, 2, S], FP8, tag="k8")
                q8_sb = io_pool.tile([64, 2, S], FP8, tag="q8")
                if h == 0:
                    # First head: small leading blocks so the first mm1
                    # chunk's inputs land early.
                    nc.sync.dma_start(k8_sb[:, :, : 2 * P], k8_d[h, :, :, : 2 * P])
                    nc.gpsimd.dma_start(q8_sb[:, :, :QW], q8_d[h, :, :, :QW])
                    nc.sync.dma_start(k8_sb[:, :, 2 * P :], k8_d[h, :, :, 2 * P :])
                    for b in range(1, QB):
                        sl = slice(b * QW, (b + 1) * QW)
                        nc.gpsimd.dma_start(q8_sb[:, :, sl], q8_d[h, :, :, sl])
                else:
                    nc.sync.dma_start(k8_sb, k8_d[h])
                    half = S // 2
                    nc.gpsimd.dma_start(q8_sb[:, :, :half], q8_d[h, :, :, :half])
                    nc.gpsimd.dma_start(q8_sb[:, :, half:], q8_d[h, :, :, half:])
                v_sb = io_pool.tile([P, KT, VW], FP16, tag="v")
                nc.sync.dma_start(v_sb, v_d[h])
                return q8_sb, k8_sb, v_sb

            def emit_phase1(h, q8_sb, k8_sb):
                # Returns pth tiles per q block: [P, KT, QW] fp16 = P'^T.
                pths = []
                for qb in range(QB):
                    tc.cur_priority = P1_BAND + h * HEAD_STRIDE + 1000 + qb * 10
                    pth = pt_pool.tile([P, KT, QW], FP16, tag="pth")
                    pths.append(pth)
                    pattern = ENG_PATTERN[qb % 2]
                    for c in range(KT // 2):
                        ps1 = ps1_pool.tile([P, 2, QW], F32, tag="ps1")
                        for j in range(2):
                            kt = 2 * c + j
                            nc.tensor.matmul(
                                ps1[:, j, :],
                                lhsT=k8_sb[:, :, kt * P : (kt + 1) * P],
                                rhs=q8_sb[:, :, qb * QW : (qb + 1) * QW],
                                start=True,
                                stop=True,
                                perf_mode=mybir.MatmulPerfMode.DoubleRow,
                            )
                        dst = pth[:, 2 * c : 2 * c + 2, :]
                        if pattern[c] == "A":
                            nc.scalar.activation(
                                dst,
                                ps1,
                                mybir.ActivationFunctionType.Exp,
                                scale=c_scale,
                                bias=bias_sb,
                            )
                        else:
                            nc.vector.tensor_scalar(
                                dst.bitcast(U16), ps1, a1024, b1024, MULT, ADD
                            )
                return pths

            def emit_phase2(h, pths, v_sb, tail=False):
                # 16 q-tile accumulation groups per head, packed 3 per PSUM
                # bank ([P, 3, 129] = 1548B): start=True only on the bank's
                # very first matmul (marks the 2KB zero region pending);
                # later groups' first writes land on still-pending bytes and
                # overwrite; stop=True only on the bank's last matmul. PE
                # stream order guarantees groups complete in sequence.
                runs = [(0, 3), (3, 3), (6, 3), (9, 3), (12, 3), (15, 1)]
                o_sbs = {}
                tail_bank = []
                for ri, (g0, glen) in enumerate(runs):
                    tc.cur_priority = (
                        P1_BAND + (h + 1) * HEAD_STRIDE + 1000 + ri * 20 + 5
                    )
                    if tail and ri >= 2:
                        # Last head: phase1 is done, the exp-ring banks are
                        # dead -- recycle each [P, 2, 512] f32 tile as two
                        # independent group-banks so the drain never waits
                        # on the 2 ps2 slots.
                        if not tail_bank:
                            big = ps1_pool.tile([P, 2, QW], F32, tag="ps1")
                            for b in range(2):
                                tail_bank.append(
                                    big[:, b, : 3 * (D + 1)].rearrange(
                                        "p (g c) -> p g c", g=3
                                    )
                                )
                        ps2 = tail_bank.pop(0)
                    else:
                        ps2 = ps2_pool.tile([P, 3, D + 1], F32, tag="ps2")
                    for gi in range(glen):
                        g = g0 + gi
                        qb, qc = divmod(g, QB)
                        if qb not in o_sbs:
                            o_sb = out_pool.tile([P, QB, D], F32, tag="o")
                            o_sbs[qb] = o_sb
                        pth = pths[qb]
                        for kt in range(KT):
                            nc.tensor.matmul(
                                ps2[:, gi, :],
                                lhsT=pth[:, kt, qc * P : (qc + 1) * P],
                                rhs=v_sb[:, kt, : D + 1],
                                start=(gi == 0 and kt == 0),
                                stop=(gi == glen - 1 and kt == KT - 1),
                                skip_group_check=True,
                            )
                    # Evacuate the bank to SBUF on DVE (frees the PSUM bank
                    # fast), reciprocal from SBUF, then scale on Pool (idle;
                    # it cannot read PSUM but can read the staged copy).
                    # Tail: scales go to the then-idle ACT.
                    stage = stage_pool.tile([P, 3, D + 1], F32, tag="st")
                    nc.vector.tensor_scalar(
                        stage[:, :glen, :], ps2[:, :glen, :], 1.0, 0.0, MULT, ADD
                    )
                    recip = small_pool.tile([P, 3, 1], F32, tag="recip")
                    nc.vector.reciprocal(
                        recip[:, :glen, :], stage[:, :glen, D : D + 1]
                    )
                    for gi in range(glen):
                        g = g0 + gi
                        qb, qc = divmod(g, QB)
                        if tail:
                            nc.scalar.mul(
                                o_sbs[qb][:, qc, :],
                                stage[:, gi, :D],
                                recip[:, gi, :],
                            )
                        else:
                            nc.gpsimd.tensor_scalar(
                                o_sbs[qb][:, qc, :],
                                stage[:, gi, :D],
                                recip[:, gi, :],
                                0.0,
                                MULT,
                                ADD,
                            )
                        if qc == QB - 1:
                            nc.sync.dma_start(
                                out_d[h, qb * QW : (qb + 1) * QW, :].rearrange(
                                    "(c p) d -> p c d", p=P
                                ),
                                o_sbs[qb],
                            )

            bias_sb = small_pool.tile([P, 1], F32, tag="bias", bufs=1)
            nc.vector.memset(bias_sb, -c_shift)

            # PE p-state warmup: the tensor engine ramps 0.65 -> 1.2 ->
            # 2.4 GHz with ~3us of continuous execution. Run throwaway
            # matmuls on a zeroed scratch tile while the first DMAs are in
            # flight so the real matmul1 stream starts at full clock.
            tc.cur_priority = P1_BAND - 1000
            warm_sb = small_pool.tile([64, 2, P], FP8, tag="warm", bufs=1)
            nc.vector.memset(warm_sb, 0.0)
            warm_ps = ps1_pool.tile([P, 2, QW], F32, tag="ps1")
            for _ in range(26):
                nc.tensor.matmul(
                    warm_ps[:, 0, :P],
                    lhsT=warm_sb,
                    rhs=warm_sb.rearrange("p a b -> p (a b)")[:, : 2 * P].rearrange(
                        "p (a b) -> p a b", a=2
                    ),
                    start=True,
                    stop=True,
                    perf_mode=mybir.MatmulPerfMode.DoubleRow,
                )

            prev = None
            for h in range(HEADS_PER_CORE):
                q8_sb, k8_sb, v_sb = emit_load(h)
                pths = emit_phase1(h, q8_sb, k8_sb)
                if prev is not None:
                    emit_phase2(*prev)
                prev = (h, pths, v_sb)
            emit_phase2(*prev, tail=True)

    nc.compile()
    return nc


def kernel(s, q, k, v, qs, ks, vs):
    global LAST_RESULTS, LAST_NC
    q = np.asarray(q, dtype=np.float32)
    k = np.asarray(k, dtype=np.float32)
    v = np.asarray(v, dtype=np.float32)
    qs = np.asarray(qs, dtype=np.float32)
    ks = np.asarray(ks, dtype=np.float32)
    vs = np.asarray(vs, dtype=np.float32)

    B, H, S_, D_ = q.shape
    assert (S_, D_) == (S, D) and B * H == N_CORES * HEADS_PER_CORE

    # [BH, S, D] -> [BH, D, S] -> [BH, 2, 64, S] -> [BH, 64, 2, S]
    def to_fp8_halves(x):
        xt = x.reshape(B * H, S, D).transpose(0, 2, 1)
        xt = xt.reshape(B * H, 2, 64, S).transpose(0, 2, 1, 3)
        return np.ascontiguousarray(xt).astype(ml_dtypes.float8_e4m3)

    q8 = to_fp8_halves(q)
    k8 = to_fp8_halves(k)

    # v pre-scaled by vs in fp16, ones column at 128, zero pad at 129,
    # laid out [BH, P, KT, VW] so each partition row is contiguous.
    vb = np.zeros((B * H, P, KT, VW), dtype=np.float16)
    vt = (v.reshape(B * H, S, D) * vs[0]).astype(np.float16)
    vb[:, :, :, :D] = vt.reshape(B * H, KT, P, D).transpose(0, 2, 1, 3)
    vb[:, :, :, D] = np.float16(1.0)

    c_scale = float(
        np.float32(qs[0]) * np.float32(ks[0]) * np.float32(1.0 / math.sqrt(D))
    )
    # Shift so each row's max lands near 1.0 (row max of S N(0,1)-ish logits
    # is ~3.7 sigma; sigma = c_scale*sqrt(D)); cancels in the division.
    c_shift = 3.7 * math.sqrt(D) * c_scale

    nc = _build_program(c_scale, c_shift)
    LAST_NC = nc

    in_maps = []
    for c in range(N_CORES):
        lo, hi = c * HEADS_PER_CORE, (c + 1) * HEADS_PER_CORE
        in_maps.append(
            {
                "q8": np.ascontiguousarray(q8[lo:hi]),
                "k8": np.ascontiguousarray(k8[lo:hi]),
                "v16": np.ascontiguousarray(vb[lo:hi]),
            }
        )

    try:
        res = run_bass_kernel_spmd(nc, in_maps, core_ids=list(range(N_CORES)))
    except ModuleNotFoundError:
        os.environ["BASS_NEVER_TRACE"] = "1"
        res = run_bass_kernel_spmd(nc, in_maps, core_ids=list(range(N_CORES)))
    LAST_RESULTS = res

    out = np.stack([r["out"] for r in res.results])  # [8, 4, S, D] f32
    return out.reshape(B, H, S, D).astype(np.float32)


# revision 37
# speedup vs baseline: 1.0315x; 1.0315x over previous
"""Trainium2 Bass kernel for nn_AttentionModel_47983374631276.

SDPA attention: B=2, H=16, S=2048, D=128, fp8-representable q/k/v with
per-tensor dequant scales (qs, ks, vs).

Sharding: batch*heads = 32 pairs -> 4 heads per core across 8 cores.
Each core runs its full S x S attention locally; no cross-core comm.

Per-head device algorithm:
  1. matmul1 in fp8e4 with DoubleRow perf mode (lossless: q/k values are
     exactly fp8-representable; D=128 contraction is split into 2 k-tiles
     of 64 partitions, PE processes both per pass at 0.5 cycles/row):
     logits^T[k, q] slices [128, 512] into PSUM f32.
  2. exp, split across three engines to beat the ACT-only roofline:
     - ACT (ScalarE): exact exp -> fp16, scale=qs*ks/sqrt(D), bias=-shift.
     - DVE + Pool: Schraudolph bit trick: one tensor_scalar
       (logit * A1024 + B1024) -> uint16 (round-nearest, saturating at 0)
       whose bits ARE the fp16 representation of exp(scale*l - shift)
       with a zero-mean multiplicative error of std 1.9%. ~44% of slices
       use the approximation; measured end-to-end rel err ~1.2e-2 < 2e-2.
  3. matmul2 (fp16): out[q, 129] = sum_kt P'^T[kt].T @ [V*vs | 1]
     (ones column gives the softmax denominator; V is pre-scaled by vs on
     the host - exactly representable in fp16 up to 2^-11 relative).
  4. evac: DVE reciprocal of the denominator, Pool tensor_scalar multiply,
     staged [128, 4, 128] f32 in SBUF, one DMA per 512 queries.

Software pipelining: phase2 of head h-1 is emitted after phase1 of head h
in a lower priority band, so the Tile scheduler fills PE gaps (ps1 ring
full) with matmul2 work while ACT/DVE/Pool drain exp chunks.
"""

import math
import os

import numpy as np
import ml_dtypes

import concourse.bacc as bacc
import concourse.bass as bass
import concourse.tile as tile
import concourse.mybir as mybir
from concourse.bass_utils import run_bass_kernel_spmd

N_CORES = 8
HEADS_PER_CORE = 4
S = 2048
D = 128
P = 128            # partitions
KT = S // P        # 16 k tiles per head
QB = 4             # q blocks of 512
QW = S // QB       # 512
VW = 130           # v row width: 128 values + ones col + pad

FP8 = mybir.dt.float8e4
BF16 = mybir.dt.bfloat16
FP16 = mybir.dt.float16
U16 = mybir.dt.uint16
F32 = mybir.dt.float32
MULT = mybir.AluOpType.mult
ADD = mybir.AluOpType.add

# exp chunk -> engine pattern per q-block (8 chunks of 2 kt each).
# 'A' = ScalarE exact exp, 'D' = DVE bit trick. (Pool/GPSIMD cannot read
# PSUM on trn2, so it can't join the exp work directly.)
ENG_PATTERN = [
    ("A", "A", "D", "A", "D", "A", "D", "A"),   # 5A 3D
    ("A", "D", "D", "A", "A", "D", "A", "D"),   # 4A 4D
]

# Schraudolph constant tuned so E[approx/exact] = 1 over the mantissa
# interval (zero-mean multiplicative error; bias would not cancel in rows
# that mix exact and approximated slices).
C0_SCHRAUDOLPH = -0.0564

# Stash of the most recent run results / program for test harnesses.
LAST_RESULTS = None
LAST_NC = None


def _build_program(c_scale: float, c_shift: float):
    nc = bacc.Bacc()

    q8_d = nc.dram_tensor("q8", [HEADS_PER_CORE, 64, 2, S], FP8, kind="ExternalInput")
    k8_d = nc.dram_tensor("k8", [HEADS_PER_CORE, 64, 2, S], FP8, kind="ExternalInput")
    v_d = nc.dram_tensor("v16", [HEADS_PER_CORE, P, KT, VW], FP16, kind="ExternalInput")
    out_d = nc.dram_tensor("out", [HEADS_PER_CORE, S, D], F32, kind="ExternalOutput")

    a1024 = float(np.float32(c_scale * math.log2(math.e) * 1024.0))
    b1024 = float(np.float32(
        (-c_shift * math.log2(math.e) + 15.0 + C0_SCHRAUDOLPH) * 1024.0
    ))

    with tile.TileContext(nc) as tc:
        with (
            tc.tile_pool(name="io", bufs=2) as io_pool,
            tc.tile_pool(name="ptp", bufs=8) as pt_pool,
            tc.tile_pool(name="outp", bufs=4) as out_pool,
            tc.tile_pool(name="stagep", bufs=4) as stage_pool,
            tc.tile_pool(name="smallp", bufs=4) as small_pool,
            tc.tile_pool(name="ps1p", bufs=3, space="PSUM") as ps1_pool,
            tc.tile_pool(name="ps2p", bufs=2, space="PSUM") as ps2_pool,
        ):
            P1_BAND = 0
            P2_BAND = 10_000_000
            HEAD_STRIDE = 100_000

            def emit_load(h):
                tc.cur_priority = P1_BAND + h * HEAD_STRIDE
                k8_sb = io_pool.tile([64, 2, S], FP8, tag="k8")
                q8_sb = io_pool.tile([64, 2, S], FP8, tag="q8")
                if h == 0:
                    # First head: small leading blocks so the first mm1
                    # chunk's inputs land early.
                    nc.sync.dma_start(k8_sb[:, :, : 2 * P], k8_d[h, :, :, : 2 * P])
                    nc.gpsimd.dma_start(q8_sb[:, :, :QW], q8_d[h, :, :, :QW])
                    nc.sync.dma_start(k8_sb[:, :, 2 * P :], k8_d[h, :, :, 2 * P :])
                    for b in range(1, QB):
                        sl = slice(b * QW, (b + 1) * QW)
                        nc.gpsimd.dma_start(q8_sb[:, :, sl], q8_d[h, :, :, sl])
                else:
                    nc.sync.dma_start(k8_sb, k8_d[h])
                    half = S // 2
                    nc.gpsimd.dma_start(q8_sb[:, :, :half], q8_d[h, :, :, :half])
                    nc.gpsimd.dma_start(q8_sb[:, :, half:], q8_d[h, :, :, half:])
                v_sb = io_pool.tile([P, KT, VW], FP16, tag="v")
                nc.sync.dma_start(v_sb, v_d[h])
                return q8_sb, k8_sb, v_sb

            def emit_phase1(h, q8_sb, k8_sb):
                # Returns pth tiles per q block: [P, KT, QW] fp16 = P'^T.
                pths = []
                for qb in range(QB):
                    tc.cur_priority = P1_BAND + h * HEAD_STRIDE + 1000 + qb * 10
                    pth = pt_pool.tile([P, KT, QW], FP16, tag="pth")
                    pths.append(pth)
                    pattern = ENG_PATTERN[qb % 2]
                    for c in range(KT // 2):
                        ps1 = ps1_pool.tile([P, 2, QW], F32, tag="ps1")
                        for j in range(2):
                            kt = 2 * c + j
                            nc.tensor.matmul(
                                ps1[:, j, :],
                                lhsT=k8_sb[:, :, kt * P : (kt + 1) * P],
                                rhs=q8_sb[:, :, qb * QW : (qb + 1) * QW],
                                start=True,
                                stop=True,
                                perf_mode=mybir.MatmulPerfMode.DoubleRow,
                            )
                        dst = pth[:, 2 * c : 2 * c + 2, :]
                        if pattern[c] == "A":
                            nc.scalar.activation(
                                dst,
                                ps1,
                                mybir.ActivationFunctionType.Exp,
                                scale=c_scale,
                                bias=bias_sb,
                            )
                        else:
                            nc.vector.tensor_scalar(
                                dst.bitcast(U16), ps1, a1024, b1024, MULT, ADD
                            )
                return pths

            def emit_phase2(h, pths, v_sb, tail=False):
                # 16 q-tile accumulation groups per head, packed 3 per PSUM
                # bank ([P, 3, 129] = 1548B): start=True only on the bank's
                # very first matmul (marks the 2KB zero region pending);
                # later groups' first writes land on still-pending bytes and
                # overwrite; stop=True only on the bank's last matmul. PE
                # stream order guarantees groups complete in sequence.
                runs = [(0, 3), (3, 3), (6, 3), (9, 3), (12, 3), (15, 1)]
                o_sbs = {}
                tail_bank = []
                for ri, (g0, glen) in enumerate(runs):
                    tc.cur_priority = (
                        P1_BAND + (h + 1) * HEAD_STRIDE + 1000 + ri * 20 + 5
                    )
                    if tail and ri >= 2:
                        # Last head: phase1 is done, the exp-ring banks are
                        # dead -- recycle each [P, 2, 512] f32 tile as two
                        # independent group-banks so the drain never waits
                        # on the 2 ps2 slots.
                        if not tail_bank:
                            big = ps1_pool.tile([P, 2, QW], F32, tag="ps1")
                            for b in range(2):
                                tail_bank.append(
                                    big[:, b, : 3 * (D + 1)].rearrange(
                                        "p (g c) -> p g c", g=3
                                    )
                                )
                        ps2 = tail_bank.pop(0)
                    else:
                        ps2 = ps2_pool.tile([P, 3, D + 1], F32, tag="ps2")
                    for gi in range(glen):
                        g = g0 + gi
                        qb, qc = divmod(g, QB)
                        if qb not in o_sbs:
                            o_sb = out_pool.tile([P, QB, D], F32, tag="o")
                            o_sbs[qb] = o_sb
                        pth = pths[qb]
                        for kt in range(KT):
                            nc.tensor.matmul(
                                ps2[:, gi, :],
                                lhsT=pth[:, kt, qc * P : (qc + 1) * P],
                                rhs=v_sb[:, kt, : D + 1],
                                start=(gi == 0 and kt == 0),
                                stop=(gi == glen - 1 and kt == KT - 1),
                                skip_group_check=True,
                            )
                    # Evacuate the bank to SBUF on DVE (frees the PSUM bank
                    # fast), reciprocal from SBUF, then scale on Pool (idle;
                    # it cannot read PSUM but can read the staged copy).
                    # Tail: scales go to the then-idle ACT.
                    stage = stage_pool.tile([P, 3, D + 1], F32, tag="st")
                    nc.vector.tensor_scalar(
                        stage[:, :glen, :], ps2[:, :glen, :], 1.0, 0.0, MULT, ADD
                    )
                    recip = small_pool.tile([P, 3, 1], F32, tag="recip")
                    nc.vector.reciprocal(
                        recip[:, :glen, :], stage[:, :glen, D : D + 1]
                    )
                    for gi in range(glen):
                        g = g0 + gi
                        qb, qc = divmod(g, QB)
                        if tail:
                            nc.scalar.mul(
                                o_sbs[qb][:, qc, :],
                                stage[:, gi, :D],
                                recip[:, gi, :],
                            )
                        else:
                            nc.gpsimd.tensor_scalar(
                                o_sbs[qb][:, qc, :],
                                stage[:, gi, :D],
                                recip[:, gi, :],
                                0.0,
                                MULT,
                                ADD,
                            )
                        if qc == QB - 1:
                            nc.sync.dma_start(
                                out_d[h, qb * QW : (qb + 1) * QW, :].rearrange(
                                    "(c p) d -> p c d", p=P
                                ),
                                o_sbs[qb],
                            )

            bias_sb = small_pool.tile([P, 1], F32, tag="bias", bufs=1)
            nc.vector.memset(bias_sb, -c_shift)

            # PE p-state warmup: the tensor engine ramps 0.65 -> 1.2 ->
            # 2.4 GHz with ~3us of continuous execution. Run throwaway
            # matmuls on a zeroed scratch tile while the first DMAs are in
            # flight so the real matmul1 stream starts at full clock.
            tc.cur_priority = P1_BAND - 1000
            warm_sb = small_pool.tile([64, 2, P], FP8, tag="warm", bufs=1)
            nc.vector.memset(warm_sb, 0.0)
            warm_ps = ps1_pool.tile([P, 2, QW], F32, tag="ps1")
            for _ in range(26):
                nc.tensor.matmul(
                    warm_ps[:, 0, :P],
                    lhsT=warm_sb,
                    rhs=warm_sb.rearrange("p a b -> p (a b)")[:, : 2 * P].rearrange(
                        "p (a b) -> p a b", a=2
                    ),
                    start=True,
                    stop=True,
                    perf_mode=mybir.MatmulPerfMode.DoubleRow,
                )

            prev = None
            for h in range(HEADS_PER_CORE):
                q8_sb, k8_sb, v_sb = emit_load(h)
                pths = emit_phase1(h, q8_sb, k8_sb)
                if prev is not None:
                    emit_phase2(*prev)
                prev = (h, pths, v_sb)
            emit_phase2(*prev, tail=True)

    nc.compile()
    return nc


def kernel(s, q, k, v, qs, ks, vs):
    global LAST_RESULTS, LAST_NC
    q = np.asarray(q, dtype=np.float32)
    k = np.asarray(k, dtype=np.float32)
    v = np.asarray(v, dtype=np.float32)
    qs = np.asarray(qs, dtype=np.float32)
    ks = np.asarray(ks, dtype=np.float32)
    vs = np.asarray(vs, dtype=np.float32)

    B, H, S_, D_ = q.shape
    assert (S_, D_) == (S, D) and B * H == N_CORES * HEADS_PER_CORE

    # [BH, S, D] -> [BH, D, S] -> [BH, 2, 64, S] -> [BH, 64, 2, S]
    def to_fp8_halves(x):
        xt = x.reshape(B * H, S, D).transpose(0, 2, 1)
        xt = xt.reshape(B * H, 2, 64, S).transpose(0, 2, 1, 3)
        return np.ascontiguousarray(xt).astype(ml_dtypes.float8_e4m3)

    q8 = to_fp8_halves(q)
    k8 = to_fp8_halves(k)

    # v pre-scaled by vs in fp16, ones column at 128, zero pad at 129,
    # laid out [BH, P, KT, VW] so each partition row is contiguous.
    vb = np.zeros((B * H, P, KT, VW), dtype=np.float16)
    vt = (v.reshape(B * H, S, D) * vs[0]).astype(np.float16)
    vb[:, :, :, :D] = vt.reshape(B * H, KT, P, D).transpose(0, 2, 1, 3)
    vb[:, :, :, D] = np.float16(1.0)

    c_scale = float(
        np.float32(qs[0]) * np.float32(ks[0]) * np.float32(1.0 / math.sqrt(D))
    )
    # Shift so each row's max lands near 1.0 (row max of S N(0,1)-ish logits
    # is ~3.7 sigma; sigma = c_scale*sqrt(D)); cancels in the division.
    c_shift = 3.7 * math.sqrt(D) * c_scale

    nc = _build_program(c_scale, c_shift)
    LAST_NC = nc

    in_maps = []
    for c in range(N_CORES):
        lo, hi = c * HEADS_PER_CORE, (c + 1) * HEADS_PER_CORE
        in_maps.append(
            {
                "q8": np.ascontiguousarray(q8[lo:hi]),
                "k8": np.ascontiguousarray(k8[lo:hi]),
                "v16": np.ascontiguousarray(vb[lo:hi]),
            }
        )

    try:
        res = run_bass_kernel_spmd(nc, in_maps, core_ids=list(range(N_CORES)))
    except ModuleNotFoundError:
        os.environ["BASS_NEVER_TRACE"] = "1"
        res = run_bass_kernel_spmd(nc, in_maps, core_ids=list(range(N_CORES)))
    LAST_RESULTS = res

    out = np.stack([r["out"] for r in res.results])  # [8, 4, S, D] f32
    return out.reshape(B, H, S, D).astype(np.float32)
